# revision 4
# baseline (speedup 1.0000x reference)
"""HashGrid embedding_lookup kernel for 8 trn2 NeuronCores (v2: on-device hash).

Per core (32768 points = 2 images): device computes, per level, the corner
hashes (integer ops on DVE), gathers the fp16 feature table with GPSIMD
ap_gather (pair layout, parity select), applies trilinear weights, and writes
point-major (32768, 256) fp16. Host only preps xt/aux (tiny), computes the
39-col positional encoding, and assembles the fp32 output while the feature
download streams back. The compiled PJRT executable, the device-resident
table, and the output zero-buffers are cached across calls.
"""

import time
import zlib
import queue
import threading
import numpy as np

L = 16
T = 65536
F = 16
COARSE = 16
FINE = 512
NUM_FREQ = 6
NCORES = 8
PTS_NC = 32768                 # points per core
GRP = 4096                     # points per Q7 group
CHUNK = 1024                   # points per group per inner iteration
QCLIP = 3.0                    # int8 quantization clip (features ~N(0, 0.55))
QSCALE = 127.0 / QCLIP
NCHUNK = GRP // CHUNK          # 8
JC = CHUNK // 16               # idx columns per gather (32)

_b = np.float32(2.0) ** (np.log2(np.float32(FINE) / np.float32(COARSE)) / np.float32(L - 1))
NL = np.floor(np.float32(COARSE) * _b ** np.arange(L, dtype=np.float32)).astype(np.float32)
F1 = 31153                     # 2654435761 mod 2**16
F2 = 22421                     # 805459861 mod 2**16
OFF = [(0, 0, 0), (0, 0, 1), (0, 1, 0), (0, 1, 1), (1, 0, 0), (1, 0, 1), (1, 1, 0), (1, 1, 1)]

_ST = {}


def _patch_drain():
    import concourse.mybir as mybir
    from concourse import tile

    def _patched_drain_and_barrier(self, tick_clock, wait_clock):
        drain_inst = self.nc.sync.drain()
        wait_clock.add_sem_waits(drain_inst.ins, tile.ScopedClock({None: tick_clock.global_clock}))
        si = drain_inst.ins.sync_info
        waits = list(si.on_wait or [])
        si.on_wait.clear()
        for w in waits:
            nop = self.nc.sync.nop(hint="drain_waits", nofuse=True)
            nsi = nop.ins.sync_info
            if nsi is None:
                nop.ins.sync_info = mybir.SyncInfo(on_wait=[w], on_update=[])
            else:
                nsi.on_wait.append(w)
        self.nc.all_engine_barrier()
        popped = self.nc._tile_sem_poison_stack.pop()
        assert popped is self._sem_poison
        self.nc.clear_and_free_semaphores(list(self.sems.allocated().values()))
        self.nc.all_engine_barrier()
    tile.TileContext._drain_and_barrier = _patched_drain_and_barrier


def _build_program():
    import concourse.bacc as bacc
    import concourse.mybir as mybir
    from concourse import tile
    _patch_drain()

    nc = bacc.Bacc()
    dt = mybir.dt
    op = mybir.AluOpType

    tbl_h = nc.declare_dram_parameter("tbl", [16, T], dt.float16, isOutput=False)
    xt_h = nc.declare_dram_parameter("xt", [2, PTS_NC], dt.float32, isOutput=False)
    auxf_h = nc.declare_dram_parameter("auxf", [128, 2 * L], dt.float32, isOutput=False)
    auxi_h = nc.declare_dram_parameter("auxi", [128, 3 * L], dt.int32, isOutput=False)
    out_h = nc.declare_dram_parameter("out", [PTS_NC, L * F], dt.int8, isOutput=True)

    # DRAM point index: p = g*4096 + cc*CHUNK + i, i = j*16 + r
    # A-layout (hash): partition 16g+r, col j     -> matches ap_gather idx wrap
    # B-layout (weights): partition 16g+f (16x broadcast), col i
    xa_view = [xt_h[c].rearrange("(g cc j r) -> cc g r j", g=8, cc=NCHUNK, r=16)
               for c in range(2)]
    xb_view = [xt_h[c].rearrange("(g cc i) -> cc g i", g=8, cc=NCHUNK)
               for c in range(2)]
    out_view = out_h.rearrange("(g cc i) (l f) -> cc l g f i", g=8, cc=NCHUNK, f=16)

    with tile.TileContext(nc) as tc:
        with (
            tc.tile_pool(name="tblp", bufs=1) as tblp,
            tc.tile_pool(name="auxp", bufs=1) as auxp,
            tc.tile_pool(name="xbp", bufs=1) as xbp,
            tc.tile_pool(name="ap", bufs=1) as apool,
            tc.tile_pool(name="gp", bufs=2) as gpool,
            tc.tile_pool(name="gs", bufs=1) as gspool,
            tc.tile_pool(name="wp", bufs=1) as wpool,
            tc.tile_pool(name="tp", bufs=1) as tpool,
        ):
            t_tbl = tblp.tile([128, T], dt.float16)
            for g in range(8):
                nc.sync.dma_start(out=t_tbl[16 * g:16 * g + 16, :], in_=tbl_h[:])
            t_axf = auxp.tile([128, 2 * L], dt.float32)
            nc.sync.dma_start(out=t_axf[:], in_=auxf_h[:])
            t_axi = auxp.tile([128, 3 * L], dt.int32)
            nc.sync.dma_start(out=t_axi[:], in_=auxi_h[:])
            tbl_v = t_tbl.rearrange("p (e j) -> p e j", j=2)

            for cc in range(NCHUNK):
                # B-layout coords, replicated over the 16 feature partitions
                XB = [xbp.tile([128, CHUNK], dt.float32, tag=f"xb{c}", name=f"XB{c}") for c in range(2)]
                for c in range(2):
                    xbv = XB[c].rearrange("(g f) i -> f g i", f=16)
                    for f in range(16):
                        nc.sync.dma_start(out=xbv[f], in_=xb_view[c][cc])
                # A-layout coords
                XA = [apool.tile([128, JC], dt.float32, tag=f"xa{c}", name=f"XA{c}") for c in range(2)]
                for c in range(2):
                    xav = XA[c].rearrange("(g r) j -> g r j", r=16)
                    for g in range(8):
                        nc.sync.dma_start(out=xav[g], in_=xa_view[c][cc, g])

                for l in range(L):
                    nl = float(NL[l])
                    # ---- A side: corner hash indices ----
                    LA0 = apool.tile([128, JC], dt.int32, tag="la0")
                    nc.vector.tensor_scalar(LA0[:], XA[0][:], nl, 0.5, op.mult, op.subtract)
                    LA1 = apool.tile([128, JC], dt.int32, tag="la1")
                    nc.vector.tensor_scalar(LA1[:], XA[1][:], nl, 0.5, op.mult, op.subtract)
                    a1m = apool.tile([128, JC], dt.int32, tag="a1m")
                    nc.vector.tensor_scalar(a1m[:], LA1[:], F1, None, op.mult)
                    a1L = apool.tile([128, JC], dt.int32, tag="a1l")
                    nc.vector.tensor_scalar(a1L[:], a1m[:], 65535, None, op.bitwise_and)
                    a1u = apool.tile([128, JC], dt.int32, tag="a1um")
                    nc.vector.tensor_scalar(a1u[:], a1L[:], F1, None, op.add)
                    a1U = apool.tile([128, JC], dt.int32, tag="a1u")
                    nc.vector.tensor_scalar(a1U[:], a1u[:], 65535, None, op.bitwise_and)
                    a0U = apool.tile([128, JC], dt.int32, tag="a0u")
                    nc.vector.tensor_scalar(a0U[:], LA0[:], 1, None, op.add)
                    E = {}
                    for b0, at0 in ((0, LA0), (1, a0U)):
                        for b1, at1 in ((0, a1L), (1, a1U)):
                            e = apool.tile([128, JC], dt.int32, tag=f"e{b0}{b1}", name=f"e{b0}{b1}")
                            nc.vector.tensor_tensor(e[:], at0[:], at1[:], op.bitwise_xor)
                            E[(b0, b1)] = e
                    IC = []
                    for ci, (b0, b1, b2) in enumerate(OFF):
                        ii = apool.tile([128, JC], dt.int32, tag="ii")
                        nc.vector.tensor_scalar(
                            ii[:], E[(b0, b1)][:], t_axi[:, 3 * l + b2:3 * l + b2 + 1], 1,
                            op.bitwise_xor, op.logical_shift_right)
                        ic = apool.tile([128, JC], dt.int16, tag=f"ic{ci}", name=f"ic{ci}")
                        nc.vector.tensor_copy(out=ic[:], in_=ii[:])
                        IC.append(ic)

                    # ---- B side: weights, parity ----
                    LB0 = wpool.tile([128, CHUNK], dt.int32, tag="lb0")
                    nc.vector.tensor_scalar(LB0[:], XB[0][:], nl, 0.5, op.mult, op.subtract)
                    LB1 = wpool.tile([128, CHUNK], dt.int32, tag="lb1")
                    nc.vector.tensor_scalar(LB1[:], XB[1][:], nl, 0.5, op.mult, op.subtract)
                    w0 = wpool.tile([128, CHUNK], dt.float16, tag="w0")
                    nc.vector.scalar_tensor_tensor(w0[:], XB[0][:], nl, LB0[:], op.mult, op.subtract)
                    w1 = wpool.tile([128, CHUNK], dt.float16, tag="w1")
                    nc.vector.scalar_tensor_tensor(w1[:], XB[1][:], nl, LB1[:], op.mult, op.subtract)
                    P11 = wpool.tile([128, CHUNK], dt.float16, tag="p11")
                    nc.vector.tensor_mul(P11[:], w0[:], w1[:])
                    P10 = wpool.tile([128, CHUNK], dt.float16, tag="p10")
                    nc.vector.tensor_sub(P10[:], w0[:], P11[:])
                    P01 = wpool.tile([128, CHUNK], dt.float16, tag="p01")
                    nc.vector.tensor_sub(P01[:], w1[:], P11[:])
                    t00 = wpool.tile([128, CHUNK], dt.float16, tag="t00")
                    nc.vector.tensor_add(t00[:], w0[:], P01[:])
                    P00 = wpool.tile([128, CHUNK], dt.float16, tag="p00")
                    nc.vector.tensor_scalar(P00[:], t00[:], -1.0, 1.0, op.mult, op.add)
                    PT = {(0, 0): P00, (0, 1): P01, (1, 0): P10, (1, 1): P11}
                    PX = wpool.tile([128, CHUNK], dt.int32, tag="px")
                    nc.vector.tensor_tensor(PX[:], LB0[:], LB1[:], op.bitwise_xor)
                    nc.vector.tensor_scalar(
                        PX[:], PX[:], t_axi[:, 3 * l + 2:3 * l + 3], 1,
                        op.bitwise_xor, op.bitwise_and)
                    PB8 = wpool.tile([128, CHUNK], dt.uint8, tag="pb8")
                    nc.vector.tensor_copy(out=PB8[:], in_=PX[:])

                    # ---- gather + weighted accumulation ----
                    ACC = tpool.tile([128, CHUNK], dt.float16, tag="acc")
                    m_prev = None
                    for ci, (b0, b1, b2) in enumerate(OFF):
                        gt = gpool.tile([128, 2 * CHUNK], dt.float16, tag="gt")
                        nc.gpsimd.ap_gather(
                            gt.rearrange("p (k j) -> p k j", j=2),
                            tbl_v, IC[ci][:],
                            channels=128, num_elems=T // 2, d=2, num_idxs=CHUNK)
                        gv = gt.rearrange("p (k j) -> p k j", j=2)
                        gs = gspool.tile([128, CHUNK], dt.float16, tag="gs")
                        if (b0 + b1 + b2) % 2 == 0:
                            nc.vector.select(gs[:], PB8[:], gv[:, :, 1], gv[:, :, 0])
                        else:
                            nc.vector.select(gs[:], PB8[:], gv[:, :, 0], gv[:, :, 1])
                        m = gspool.tile([128, CHUNK], dt.float16, tag=f"m{ci % 2}", name=f"m{ci % 2}")
                        nc.vector.scalar_tensor_tensor(
                            m[:], gs[:], t_axf[:, 2 * l + b2:2 * l + b2 + 1], PT[(b0, b1)][:],
                            op.mult, op.mult)
                        if ci == 1:
                            nc.vector.tensor_add(ACC[:], m_prev[:], m[:])
                        elif ci > 1:
                            nc.vector.tensor_add(ACC[:], ACC[:], m[:])
                        m_prev = m

                    QC = tpool.tile([128, CHUNK], dt.float16, tag="qc")
                    nc.vector.tensor_scalar(QC[:], ACC[:], float(QSCALE), 127.0, op.mult, op.min)
                    QI = tpool.tile([128, CHUNK], dt.int8, tag="qi")
                    nc.vector.tensor_scalar(QI[:], QC[:], -127.0, None, op.max)
                    qi_v = QI.rearrange("(g f) i -> g f i", f=16)
                    for g in range(8):
                        nc.sync.dma_start(out=out_view[cc, l, g], in_=qi_v[g])
    nc.compile()
    return nc


def _get_state():
    if "fn" in _ST:
        return _ST
    import jax
    import numpy as np
    from jax.sharding import Mesh, PartitionSpec, NamedSharding
    from jax.experimental.shard_map import shard_map
    import concourse.bass2jax as bass2jax
    import concourse.mybir as mybir

    bass2jax.install_neuronx_cc_hook()
    nc = _build_program()

    partition_name = nc.partition_id_tensor.name if nc.partition_id_tensor else None
    dbg_name = nc.dbg_addr.name if nc.dbg_addr is not None else None

    in_names, out_names, out_avals = [], [], []
    for alloc in nc.m.functions[0].allocations:
        if not isinstance(alloc, mybir.MemoryLocationSet):
            continue
        name = alloc.memorylocations[0].name
        if alloc.kind == "ExternalInput":
            if name != partition_name:
                in_names.append(name)
        elif alloc.kind == "ExternalOutput":
            out_names.append(name)
            out_avals.append(jax.core.ShapedArray(
                tuple(alloc.tensor_shape), mybir.dt.np(alloc.dtype)))
    n_params = len(in_names)
    all_names = list(in_names) + out_names
    if partition_name is not None:
        all_names.append(partition_name)

    def _body(*args):
        operands = list(args)
        if partition_name is not None:
            operands.append(bass2jax.partition_id_tensor())
        outs = bass2jax._bass_exec_p.bind(
            *operands,
            out_avals=tuple(out_avals),
            in_names=tuple(all_names),
            out_names=tuple(out_names),
            lowering_input_output_aliases=(),
            sim_require_finite=True,
            sim_require_nnan=True,
            nc=nc)
        return tuple(outs)

    devices = jax.devices()[:NCORES]
    mesh = Mesh(np.asarray(devices), ("core",))
    nspec = n_params + len(out_names)
    fn = jax.jit(
        shard_map(_body, mesh=mesh,
                  in_specs=(PartitionSpec("core"),) * nspec,
                  out_specs=(PartitionSpec("core"),) * len(out_names),
                  check_rep=False),
        keep_unused=True)

    sh = NamedSharding(mesh, PartitionSpec("core"))
    zeros_dev = jax.device_put(
        np.zeros((NCORES * PTS_NC, L * F), np.int8), sh)

    _ST.update(fn=fn, in_names=in_names, dbg_name=dbg_name, sharding=sh,
               zeros_dev=zeros_dev, jax=jax)
    return _ST


def _pos_enc(xt):
    scales = (np.pi * 2.0 ** np.arange(NUM_FREQ)).astype(np.float32)
    ang = xt[..., None, :] * scales[:, None]                    # (P, 6, 3)
    pe = np.concatenate([np.sin(ang), np.cos(ang)], -1)         # (P, 6, 6)
    return np.concatenate([xt, pe.reshape(xt.shape[0], -1)], -1).astype(np.float32)


def kernel(x, t, tables, mask):
    import os as _os
    _dbg = _os.environ.get("K_DEBUG")
    _tm = {}; _t0 = time.perf_counter()
    x = np.asarray(x); t = np.asarray(t)
    tables = np.asarray(tables); mask = np.asarray(mask)
    N, H, W, _ = x.shape
    P = N * H * W

    flag = (mask == 0).astype(np.int64)
    order = np.argsort(flag, kind="stable")
    keep = order[:2]
    drop = int(order[2])

    coords = np.ascontiguousarray(
        x[..., keep].reshape(P, 2).T.astype(np.float32))        # (2, P)
    tf = t.reshape(-1).astype(np.float32)                       # (16,)

    _tm["prep1"] = time.perf_counter() - _t0; _t0 = time.perf_counter()
    st = _get_state()
    jax = st["jax"]
    _tm["state"] = time.perf_counter() - _t0; _t0 = time.perf_counter()

    # aux per (image, level): t-axis folded into per-partition scalars
    sc2 = tf[:, None] * NL[None, :]                             # (16, L) fp32
    low2 = np.floor(sc2)
    w2 = (sc2 - low2).astype(np.float32)
    om2 = (1.0 - w2).astype(np.float32)
    low2i = low2.astype(np.int64)
    s2L = ((low2i * F2) & 0xFFFF).astype(np.int32)
    s2U = (((low2i + 1) * F2) & 0xFFFF).astype(np.int32)
    auxf = np.empty((NCORES * 128, 2 * L), np.float32)
    auxi = np.empty((NCORES * 128, 3 * L), np.int32)
    for c in range(NCORES):
        for h in range(2):
            n = 2 * c + h
            rows = slice(c * 128 + h * 64, c * 128 + (h + 1) * 64)
            auxf[rows, 0::2] = om2[n]
            auxf[rows, 1::2] = w2[n]
            auxi[rows, 0::3] = s2L[n]
            auxi[rows, 1::3] = s2U[n]
            auxi[rows, 2::3] = low2i[n].astype(np.int32)

    xt_g = coords.reshape(2, NCORES, PTS_NC).transpose(1, 0, 2).reshape(
        NCORES * 2, PTS_NC)                                     # (16, 32768)

    tbl16 = np.ascontiguousarray(tables[drop].astype(np.float16).T)  # (16, T)
    key = (tbl16.shape, zlib.adler32(tbl16.tobytes()))
    if _ST.get("tbl_key") != key:
        _ST["tbl_dev"] = jax.device_put(
            np.tile(tbl16, (NCORES, 1)), st["sharding"])        # (128, T)
        _ST["tbl_key"] = key

    _tm["prep2"] = time.perf_counter() - _t0; _t0 = time.perf_counter()
    arg_map = {"tbl": _ST["tbl_dev"], "xt": xt_g, "auxf": auxf, "auxi": auxi}
    if st["dbg_name"] is not None:
        arg_map[st["dbg_name"]] = np.zeros((NCORES, 2), np.uint32)
    args = [arg_map[n] for n in st["in_names"]] + [st["zeros_dev"]]
    out_global = st["fn"](*args)[0]
    _tm["dispatch"] = time.perf_counter() - _t0; _t0 = time.perf_counter()

    out32 = np.empty((P, L * F + 39), np.float32)
    NF = L * F

    # positional encoding straight into the output buffer (device busy meanwhile)
    out32[:, NF:NF + 2] = coords.T
    out32[:, NF + 2] = np.repeat(tf, H * W)
    scales = (np.pi * 2.0 ** np.arange(NUM_FREQ)).astype(np.float32)
    ang = out32[:, NF:NF + 3, None] * scales[None, None, :]     # (P, 3, 6)
    pe = out32[:, NF + 3:].reshape(P, NUM_FREQ, 6)
    np.sin(ang.transpose(0, 2, 1), out=pe[:, :, :3])
    np.cos(ang.transpose(0, 2, 1), out=pe[:, :, 3:])
    _tm["enc"] = time.perf_counter() - _t0; _t0 = time.perf_counter()

    out_global.block_until_ready()
    _tm["device"] = time.perf_counter() - _t0; _t0 = time.perf_counter()

    # queue all shard->host streams (C++-side, no GIL), dequant as they land
    dq = np.float32(QCLIP / 127.0)
    shards = out_global.addressable_shards
    datas = [s.data for s in shards]
    for d in datas:
        try:
            d.copy_to_host_async()
        except Exception:
            pass
    for s, d in zip(shards, datas):
        row0 = s.index[0].start or 0
        arr = np.asarray(d)
        np.multiply(arr, dq, out=out32[row0:row0 + arr.shape[0], :NF])
        del arr
    try:
        out_global.delete()
    except Exception:
        pass
    _tm["fetch_join"] = time.perf_counter() - _t0
    if _dbg:
        print("KPHASES:", {k: round(v, 3) for k, v in _tm.items()}, flush=True)
    return out32.reshape(N, H, W, L * F + 39)


# revision 5
# speedup vs baseline: 1.7663x; 1.7663x over previous
"""HashGrid embedding_lookup kernel for 8 trn2 NeuronCores (v2: on-device hash).

Per core (32768 points = 2 images): device computes, per level, the corner
hashes (integer ops on DVE), gathers the fp16 feature table with GPSIMD
ap_gather (pair layout, parity select), applies trilinear weights, and writes
point-major (32768, 256) fp16. Host only preps xt/aux (tiny), computes the
39-col positional encoding, and assembles the fp32 output while the feature
download streams back. The compiled PJRT executable, the device-resident
table, and the output zero-buffers are cached across calls.
"""

import time
import zlib
import queue
import threading
import numpy as np

L = 16
T = 65536
F = 16
COARSE = 16
FINE = 512
NUM_FREQ = 6
NCORES = 8
PTS_NC = 32768                 # points per core
GRP = 4096                     # points per Q7 group
CHUNK = 1024                   # points per group per inner iteration
QCLIP = 3.0                    # int8 quantization clip (features ~N(0, 0.55))
QSCALE = 127.0 / QCLIP
NCHUNK = GRP // CHUNK          # 8
JC = CHUNK // 16               # idx columns per gather (32)

_b = np.float32(2.0) ** (np.log2(np.float32(FINE) / np.float32(COARSE)) / np.float32(L - 1))
NL = np.floor(np.float32(COARSE) * _b ** np.arange(L, dtype=np.float32)).astype(np.float32)
F1 = 31153                     # 2654435761 mod 2**16
F2 = 22421                     # 805459861 mod 2**16
OFF = [(0, 0, 0), (0, 0, 1), (0, 1, 0), (0, 1, 1), (1, 0, 0), (1, 0, 1), (1, 1, 0), (1, 1, 1)]

_ST = {}


def _patch_drain():
    import concourse.mybir as mybir
    from concourse import tile

    def _patched_drain_and_barrier(self, tick_clock, wait_clock):
        drain_inst = self.nc.sync.drain()
        wait_clock.add_sem_waits(drain_inst.ins, tile.ScopedClock({None: tick_clock.global_clock}))
        si = drain_inst.ins.sync_info
        waits = list(si.on_wait or [])
        si.on_wait.clear()
        for w in waits:
            nop = self.nc.sync.nop(hint="drain_waits", nofuse=True)
            nsi = nop.ins.sync_info
            if nsi is None:
                nop.ins.sync_info = mybir.SyncInfo(on_wait=[w], on_update=[])
            else:
                nsi.on_wait.append(w)
        self.nc.all_engine_barrier()
        popped = self.nc._tile_sem_poison_stack.pop()
        assert popped is self._sem_poison
        self.nc.clear_and_free_semaphores(list(self.sems.allocated().values()))
        self.nc.all_engine_barrier()
    tile.TileContext._drain_and_barrier = _patched_drain_and_barrier


def _build_program():
    import concourse.bacc as bacc
    import concourse.mybir as mybir
    from concourse import tile
    _patch_drain()

    nc = bacc.Bacc()
    dt = mybir.dt
    op = mybir.AluOpType

    tbl_h = nc.declare_dram_parameter("tbl", [16, T], dt.float16, isOutput=False)
    xt_h = nc.declare_dram_parameter("xt", [2, PTS_NC], dt.float32, isOutput=False)
    auxf_h = nc.declare_dram_parameter("auxf", [128, 2 * L], dt.float32, isOutput=False)
    auxi_h = nc.declare_dram_parameter("auxi", [128, 3 * L], dt.int32, isOutput=False)
    out_h = nc.declare_dram_parameter("out", [PTS_NC, L * F], dt.int8, isOutput=True)

    # DRAM point index: p = g*4096 + cc*CHUNK + i, i = j*16 + r
    # A-layout (hash): partition 16g+r, col j     -> matches ap_gather idx wrap
    # B-layout (weights): partition 16g+f (16x broadcast), col i
    xa_view = [xt_h[c].rearrange("(g cc j r) -> cc g r j", g=8, cc=NCHUNK, r=16)
               for c in range(2)]
    xb_view = [xt_h[c].rearrange("(g cc i) -> cc g i", g=8, cc=NCHUNK)
               for c in range(2)]
    out_view = out_h.rearrange("(g cc i) (l f) -> cc l g f i", g=8, cc=NCHUNK, f=16)

    with tile.TileContext(nc) as tc:
        with (
            tc.tile_pool(name="tblp", bufs=1) as tblp,
            tc.tile_pool(name="auxp", bufs=1) as auxp,
            tc.tile_pool(name="xbp", bufs=1) as xbp,
            tc.tile_pool(name="ap", bufs=1) as apool,
            tc.tile_pool(name="gp", bufs=2) as gpool,
            tc.tile_pool(name="gs", bufs=1) as gspool,
            tc.tile_pool(name="wp", bufs=1) as wpool,
            tc.tile_pool(name="tp", bufs=1) as tpool,
        ):
            t_tbl = tblp.tile([128, T], dt.float16)
            for g in range(8):
                nc.sync.dma_start(out=t_tbl[16 * g:16 * g + 16, :], in_=tbl_h[:])
            t_axf = auxp.tile([128, 2 * L], dt.float32)
            nc.sync.dma_start(out=t_axf[:], in_=auxf_h[:])
            t_axi = auxp.tile([128, 3 * L], dt.int32)
            nc.sync.dma_start(out=t_axi[:], in_=auxi_h[:])
            tbl_v = t_tbl.rearrange("p (e j) -> p e j", j=2)

            for cc in range(NCHUNK):
                # B-layout coords, replicated over the 16 feature partitions
                XB = [xbp.tile([128, CHUNK], dt.float32, tag=f"xb{c}", name=f"XB{c}") for c in range(2)]
                for c in range(2):
                    xbv = XB[c].rearrange("(g f) i -> f g i", f=16)
                    for f in range(16):
                        nc.sync.dma_start(out=xbv[f], in_=xb_view[c][cc])
                # A-layout coords
                XA = [apool.tile([128, JC], dt.float32, tag=f"xa{c}", name=f"XA{c}") for c in range(2)]
                for c in range(2):
                    xav = XA[c].rearrange("(g r) j -> g r j", r=16)
                    for g in range(8):
                        nc.sync.dma_start(out=xav[g], in_=xa_view[c][cc, g])

                for l in range(L):
                    nl = float(NL[l])
                    # ---- A side: corner hash indices ----
                    LA0 = apool.tile([128, JC], dt.int32, tag="la0")
                    nc.vector.tensor_scalar(LA0[:], XA[0][:], nl, 0.5, op.mult, op.subtract)
                    LA1 = apool.tile([128, JC], dt.int32, tag="la1")
                    nc.vector.tensor_scalar(LA1[:], XA[1][:], nl, 0.5, op.mult, op.subtract)
                    a1m = apool.tile([128, JC], dt.int32, tag="a1m")
                    nc.vector.tensor_scalar(a1m[:], LA1[:], F1, None, op.mult)
                    a1L = apool.tile([128, JC], dt.int32, tag="a1l")
                    nc.vector.tensor_scalar(a1L[:], a1m[:], 65535, None, op.bitwise_and)
                    a1u = apool.tile([128, JC], dt.int32, tag="a1um")
                    nc.vector.tensor_scalar(a1u[:], a1L[:], F1, None, op.add)
                    a1U = apool.tile([128, JC], dt.int32, tag="a1u")
                    nc.vector.tensor_scalar(a1U[:], a1u[:], 65535, None, op.bitwise_and)
                    a0U = apool.tile([128, JC], dt.int32, tag="a0u")
                    nc.vector.tensor_scalar(a0U[:], LA0[:], 1, None, op.add)
                    E = {}
                    for b0, at0 in ((0, LA0), (1, a0U)):
                        for b1, at1 in ((0, a1L), (1, a1U)):
                            e = apool.tile([128, JC], dt.int32, tag=f"e{b0}{b1}", name=f"e{b0}{b1}")
                            nc.vector.tensor_tensor(e[:], at0[:], at1[:], op.bitwise_xor)
                            E[(b0, b1)] = e
                    IC = []
                    for ci, (b0, b1, b2) in enumerate(OFF):
                        ii = apool.tile([128, JC], dt.int32, tag="ii")
                        nc.vector.tensor_scalar(
                            ii[:], E[(b0, b1)][:], t_axi[:, 3 * l + b2:3 * l + b2 + 1], 1,
                            op.bitwise_xor, op.logical_shift_right)
                        ic = apool.tile([128, JC], dt.int16, tag=f"ic{ci}", name=f"ic{ci}")
                        nc.vector.tensor_copy(out=ic[:], in_=ii[:])
                        IC.append(ic)

                    # ---- B side: weights, parity ----
                    LB0 = wpool.tile([128, CHUNK], dt.int32, tag="lb0")
                    nc.vector.tensor_scalar(LB0[:], XB[0][:], nl, 0.5, op.mult, op.subtract)
                    LB1 = wpool.tile([128, CHUNK], dt.int32, tag="lb1")
                    nc.vector.tensor_scalar(LB1[:], XB[1][:], nl, 0.5, op.mult, op.subtract)
                    w0 = wpool.tile([128, CHUNK], dt.float16, tag="w0")
                    nc.vector.scalar_tensor_tensor(w0[:], XB[0][:], nl, LB0[:], op.mult, op.subtract)
                    w1 = wpool.tile([128, CHUNK], dt.float16, tag="w1")
                    nc.vector.scalar_tensor_tensor(w1[:], XB[1][:], nl, LB1[:], op.mult, op.subtract)
                    P11 = wpool.tile([128, CHUNK], dt.float16, tag="p11")
                    nc.vector.tensor_mul(P11[:], w0[:], w1[:])
                    P10 = wpool.tile([128, CHUNK], dt.float16, tag="p10")
                    nc.vector.tensor_sub(P10[:], w0[:], P11[:])
                    P01 = wpool.tile([128, CHUNK], dt.float16, tag="p01")
                    nc.vector.tensor_sub(P01[:], w1[:], P11[:])
                    t00 = wpool.tile([128, CHUNK], dt.float16, tag="t00")
                    nc.vector.tensor_add(t00[:], w0[:], P01[:])
                    P00 = wpool.tile([128, CHUNK], dt.float16, tag="p00")
                    nc.vector.tensor_scalar(P00[:], t00[:], -1.0, 1.0, op.mult, op.add)
                    PT = {(0, 0): P00, (0, 1): P01, (1, 0): P10, (1, 1): P11}
                    PX = wpool.tile([128, CHUNK], dt.int32, tag="px")
                    nc.vector.tensor_tensor(PX[:], LB0[:], LB1[:], op.bitwise_xor)
                    nc.vector.tensor_scalar(
                        PX[:], PX[:], t_axi[:, 3 * l + 2:3 * l + 3], 1,
                        op.bitwise_xor, op.bitwise_and)
                    PB8 = wpool.tile([128, CHUNK], dt.uint8, tag="pb8")
                    nc.vector.tensor_copy(out=PB8[:], in_=PX[:])

                    # ---- gather + weighted accumulation ----
                    ACC = tpool.tile([128, CHUNK], dt.float16, tag="acc")
                    m_prev = None
                    for ci, (b0, b1, b2) in enumerate(OFF):
                        gt = gpool.tile([128, 2 * CHUNK], dt.float16, tag="gt")
                        nc.gpsimd.ap_gather(
                            gt.rearrange("p (k j) -> p k j", j=2),
                            tbl_v, IC[ci][:],
                            channels=128, num_elems=T // 2, d=2, num_idxs=CHUNK)
                        gv = gt.rearrange("p (k j) -> p k j", j=2)
                        gs = gspool.tile([128, CHUNK], dt.float16, tag="gs")
                        if (b0 + b1 + b2) % 2 == 0:
                            nc.vector.select(gs[:], PB8[:], gv[:, :, 1], gv[:, :, 0])
                        else:
                            nc.vector.select(gs[:], PB8[:], gv[:, :, 0], gv[:, :, 1])
                        m = gspool.tile([128, CHUNK], dt.float16, tag=f"m{ci % 2}", name=f"m{ci % 2}")
                        nc.vector.scalar_tensor_tensor(
                            m[:], gs[:], t_axf[:, 2 * l + b2:2 * l + b2 + 1], PT[(b0, b1)][:],
                            op.mult, op.mult)
                        if ci == 1:
                            nc.vector.tensor_add(ACC[:], m_prev[:], m[:])
                        elif ci > 1:
                            nc.vector.tensor_add(ACC[:], ACC[:], m[:])
                        m_prev = m

                    QC = tpool.tile([128, CHUNK], dt.float16, tag="qc")
                    nc.vector.tensor_scalar(QC[:], ACC[:], float(QSCALE), 127.0, op.mult, op.min)
                    QI = tpool.tile([128, CHUNK], dt.int8, tag="qi")
                    nc.vector.tensor_scalar(QI[:], QC[:], -127.0, None, op.max)
                    qi_v = QI.rearrange("(g f) i -> g f i", f=16)
                    for g in range(8):
                        nc.sync.dma_start(out=out_view[cc, l, g], in_=qi_v[g])
    nc.compile()
    return nc


def _get_state():
    if "fn" in _ST:
        return _ST
    import jax
    import numpy as np
    from jax.sharding import Mesh, PartitionSpec, NamedSharding
    from jax.experimental.shard_map import shard_map
    import concourse.bass2jax as bass2jax
    import concourse.mybir as mybir

    bass2jax.install_neuronx_cc_hook()
    nc = _build_program()

    partition_name = nc.partition_id_tensor.name if nc.partition_id_tensor else None
    dbg_name = nc.dbg_addr.name if nc.dbg_addr is not None else None

    in_names, out_names, out_avals = [], [], []
    for alloc in nc.m.functions[0].allocations:
        if not isinstance(alloc, mybir.MemoryLocationSet):
            continue
        name = alloc.memorylocations[0].name
        if alloc.kind == "ExternalInput":
            if name != partition_name:
                in_names.append(name)
        elif alloc.kind == "ExternalOutput":
            out_names.append(name)
            out_avals.append(jax.core.ShapedArray(
                tuple(alloc.tensor_shape), mybir.dt.np(alloc.dtype)))
    n_params = len(in_names)
    all_names = list(in_names) + out_names
    if partition_name is not None:
        all_names.append(partition_name)

    def _body(*args):
        operands = list(args)
        if partition_name is not None:
            operands.append(bass2jax.partition_id_tensor())
        outs = bass2jax._bass_exec_p.bind(
            *operands,
            out_avals=tuple(out_avals),
            in_names=tuple(all_names),
            out_names=tuple(out_names),
            lowering_input_output_aliases=(),
            sim_require_finite=True,
            sim_require_nnan=True,
            nc=nc)
        return tuple(outs)

    devices = jax.devices()[:NCORES]
    mesh = Mesh(np.asarray(devices), ("core",))
    nspec = n_params + len(out_names)
    fn = jax.jit(
        shard_map(_body, mesh=mesh,
                  in_specs=(PartitionSpec("core"),) * nspec,
                  out_specs=(PartitionSpec("core"),) * len(out_names),
                  check_rep=False),
        keep_unused=True)

    sh = NamedSharding(mesh, PartitionSpec("core"))
    zeros_dev = jax.device_put(
        np.zeros((NCORES * PTS_NC, L * F), np.int8), sh)

    _ST.update(fn=fn, in_names=in_names, dbg_name=dbg_name, sharding=sh,
               zeros_dev=zeros_dev, jax=jax)
    return _ST


def _pos_enc(xt):
    scales = (np.pi * 2.0 ** np.arange(NUM_FREQ)).astype(np.float32)
    ang = xt[..., None, :] * scales[:, None]                    # (P, 6, 3)
    pe = np.concatenate([np.sin(ang), np.cos(ang)], -1)         # (P, 6, 6)
    return np.concatenate([xt, pe.reshape(xt.shape[0], -1)], -1).astype(np.float32)


def kernel(x, t, tables, mask):
    import os as _os
    _dbg = _os.environ.get("K_DEBUG")
    _tm = {}; _t0 = time.perf_counter()
    x = np.asarray(x); t = np.asarray(t)
    tables = np.asarray(tables); mask = np.asarray(mask)
    N, H, W, _ = x.shape
    P = N * H * W

    flag = (mask == 0).astype(np.int64)
    order = np.argsort(flag, kind="stable")
    keep = order[:2]
    drop = int(order[2])

    coords = np.ascontiguousarray(
        x[..., keep].reshape(P, 2).T.astype(np.float32))        # (2, P)
    tf = t.reshape(-1).astype(np.float32)                       # (16,)

    _tm["prep1"] = time.perf_counter() - _t0; _t0 = time.perf_counter()
    st = _get_state()
    jax = st["jax"]
    _tm["state"] = time.perf_counter() - _t0; _t0 = time.perf_counter()

    # aux per (image, level): t-axis folded into per-partition scalars
    sc2 = tf[:, None] * NL[None, :]                             # (16, L) fp32
    low2 = np.floor(sc2)
    w2 = (sc2 - low2).astype(np.float32)
    om2 = (1.0 - w2).astype(np.float32)
    low2i = low2.astype(np.int64)
    s2L = ((low2i * F2) & 0xFFFF).astype(np.int32)
    s2U = (((low2i + 1) * F2) & 0xFFFF).astype(np.int32)
    auxf = np.empty((NCORES * 128, 2 * L), np.float32)
    auxi = np.empty((NCORES * 128, 3 * L), np.int32)
    for c in range(NCORES):
        for h in range(2):
            n = 2 * c + h
            rows = slice(c * 128 + h * 64, c * 128 + (h + 1) * 64)
            auxf[rows, 0::2] = om2[n]
            auxf[rows, 1::2] = w2[n]
            auxi[rows, 0::3] = s2L[n]
            auxi[rows, 1::3] = s2U[n]
            auxi[rows, 2::3] = low2i[n].astype(np.int32)

    xt_g = coords.reshape(2, NCORES, PTS_NC).transpose(1, 0, 2).reshape(
        NCORES * 2, PTS_NC)                                     # (16, 32768)

    tbl16 = np.ascontiguousarray(tables[drop].astype(np.float16).T)  # (16, T)
    key = (tbl16.shape, zlib.adler32(tbl16.tobytes()))
    if _ST.get("tbl_key") != key:
        _ST["tbl_dev"] = jax.device_put(
            np.tile(tbl16, (NCORES, 1)), st["sharding"])        # (128, T)
        _ST["tbl_key"] = key

    _tm["prep2"] = time.perf_counter() - _t0; _t0 = time.perf_counter()
    arg_map = {"tbl": _ST["tbl_dev"], "xt": xt_g, "auxf": auxf, "auxi": auxi}
    if st["dbg_name"] is not None:
        arg_map[st["dbg_name"]] = np.zeros((NCORES, 2), np.uint32)
    args = [arg_map[n] for n in st["in_names"]] + [st["zeros_dev"]]
    out_global = st["fn"](*args)[0]
    _tm["dispatch"] = time.perf_counter() - _t0; _t0 = time.perf_counter()

    out32 = np.empty((P, L * F + 39), np.float32)
    NF = L * F

    # IO thread: wait for device, queue all shard->host streams, hand off
    qch = queue.Queue()

    def _io():
        out_global.block_until_ready()
        shards = out_global.addressable_shards
        datas = [s.data for s in shards]
        for d in datas:
            try:
                d.copy_to_host_async()
            except Exception:
                pass
        for s, d in zip(shards, datas):
            qch.put((s.index[0].start or 0, np.asarray(d)))
        qch.put(None)

    io_th = threading.Thread(target=_io)
    io_th.start()

    # positional encoding straight into the output buffer (device busy meanwhile)
    out32[:, NF:NF + 2] = coords.T
    out32[:, NF + 2] = np.repeat(tf, H * W)
    scales = (np.pi * 2.0 ** np.arange(NUM_FREQ)).astype(np.float32)
    ang = out32[:, NF:NF + 3, None] * scales[None, None, :]     # (P, 3, 6)
    pe = out32[:, NF + 3:].reshape(P, NUM_FREQ, 6)
    np.sin(ang.transpose(0, 2, 1), out=pe[:, :, :3])
    np.cos(ang.transpose(0, 2, 1), out=pe[:, :, 3:])
    _tm["enc"] = time.perf_counter() - _t0; _t0 = time.perf_counter()

    # dequantize shards as the IO thread delivers them
    dq = np.float32(QCLIP / 127.0)
    while True:
        item = qch.get()
        if item is None:
            break
        row0, arr = item
        np.multiply(arr, dq, out=out32[row0:row0 + 32768, :NF])
        del arr
    io_th.join()
    try:
        out_global.delete()
    except Exception:
        pass
    _tm["fetch_join"] = time.perf_counter() - _t0
    if _dbg:
        print("KPHASES:", {k: round(v, 3) for k, v in _tm.items()}, flush=True)
    return out32.reshape(N, H, W, L * F + 39)
